# revision 11
# baseline (speedup 1.0000x reference)
"""GQA attention layer (B=2, L=2048, D=4096, H=32, KH=8, HD=128) on 8 TRN2 cores.

Sharding: tensor-parallel over KV heads (1 kv head + 4 q heads per core).
All matmul operands in bf16 (fp32 streams at half rate on TRN2 PE and gets no
FWL weight loads). Per core: QKV projection (x^T-tile stationary), fused
per-head RMSNorm + RoPE via host-precomputed cc/ss tables (norm weight and
softmax scale folded in), PE transposes into resident SBUF qT/kT, flash-style
attention in S^T layout with causal block skipping, per-head AllToAll (4
collectives, pipelined against attention + output projection), then the output
projection for this core's 512-token slice. Host assembles the 8 token slices.
"""
import numpy as np
import ml_dtypes

import concourse.bass as bass
import concourse.mybir as mybir
import concourse.tile as tile
from concourse import bacc
from concourse.bass_utils import run_bass_kernel_spmd

F32 = mybir.dt.float32
F32R = mybir.dt.float32r
BF = mybir.dt.bfloat16
BF16NP = ml_dtypes.bfloat16
AF = mybir.ActivationFunctionType
MUL = mybir.AluOpType.mult
ADD = mybir.AluOpType.add
X_AX = mybir.AxisListType.X

B, L, D = 2, 2048, 4096
H, KH, HD = 32, 8, 128
T = B * L              # 4096 tokens
NC_ = 8                # cores
QH = H // NC_          # 4 q heads per core
QB = 512               # q block
NT = T // 128          # 32 token tiles
EPS = 1e-5
ROPE_BASE = 1000000.0

_CACHE = {}


def _build():
    nc = bacc.Bacc("TRN2", target_bir_lowering=False, debug=False, num_devices=NC_)

    xT = nc.dram_tensor("xT", [D, T], BF, kind="ExternalInput").ap()
    wq = nc.dram_tensor("wq", [D, QH * HD], BF, kind="ExternalInput").ap()
    wkv = nc.dram_tensor("wkv", [D, 2 * HD], BF, kind="ExternalInput").ap()
    wo = nc.dram_tensor("wo", [D, D], BF, kind="ExternalInput").ap()
    ropecc = nc.dram_tensor("ropecc", [T, 5 * HD], BF, kind="ExternalInput").ap()
    ropess = nc.dram_tensor("ropess", [T, 5 * HD], BF, kind="ExternalInput").ap()
    pat = nc.dram_tensor("pat", [128, 896], BF, kind="ExternalInput").ap()
    ident = nc.dram_tensor("ident", [128, 128], BF, kind="ExternalInput").ap()
    ones_c = nc.dram_tensor("ones_c", [128, 1], BF, kind="ExternalInput").ap()
    ones_r = nc.dram_tensor("ones_r", [1, 128], BF, kind="ExternalInput").ap()
    out = nc.dram_tensor("out", [T // NC_, D], F32, kind="ExternalOutput").ap()

    xT_r = xT.rearrange("(o p) t -> p o t", p=128)         # [128, 32, T]
    wq_r = wq.rearrange("(o p) n -> p o n", p=128)         # [128, 32, 512]
    wkv_r = wkv.rearrange("(o p) n -> p o n", p=128)       # [128, 32, 256]
    wo_r = wo.rearrange("(o p) n -> p o n", p=128)         # [128, 32, 4096]
    rcc_r = ropecc.rearrange("(n p) (c j) -> p n c j", p=128, j=128)  # [128,32,5,128]
    rss_r = ropess.rearrange("(n p) (c j) -> p n c j", p=128, j=128)

    with tile.TileContext(nc) as tc:
        with (
            tc.tile_pool(name="const", bufs=1) as cp,
            tc.tile_pool(name="dram", bufs=1, space="DRAM") as dramp,
            tc.tile_pool(name="res", bufs=1) as resp,
        ):
            ident_sb = cp.tile([128, 128], BF)
            nc.sync.dma_start(ident_sb[:], ident)
            pat_sb = cp.tile([128, 896], BF)
            nc.sync.dma_start(pat_sb[:], pat)
            onesc_sb = cp.tile([128, 1], BF)
            nc.sync.dma_start(onesc_sb[:], ones_c)
            onesr_sb = cp.tile([1, 128], BF)
            nc.sync.dma_start(onesr_sb[:], ones_r)

            kT_sb = resp.tile([128, T], BF)                # [hd, tok]
            v_sb = resp.tile([128, NT, HD], BF)            # [tok%128, tile, hd]
            qT_sb = resp.tile([128, QH, T], BF)            # [hd, head, tok]

            a2a_in = [dramp.tile([NC_, HD, QB], BF, name=f"a2a_in{h}")
                      for h in range(QH)]
            a2a_out = [dramp.tile([NC_, HD, QB], BF, name=f"a2a_out{h}")
                       for h in range(QH)]

            # ---------------- phase 1: projections + norm + rope ----------
            with (
                tc.tile_pool(name="wts", bufs=1) as wp,
                tc.tile_pool(name="px", bufs=2) as px,
                tc.tile_pool(name="p1", bufs=2) as p1,
                tc.tile_pool(name="ps1", bufs=2, space="PSUM") as ps1,
                tc.tile_pool(name="pst", bufs=2, space="PSUM") as pst,
            ):
                wq_sb = wp.tile([128, 32, QH * HD], BF)
                wkv_sb = wp.tile([128, 32, 2 * HD], BF)
                # first x/table tiles ahead of the big weight DMAs so the
                # tensor engine starts as soon as weight chunk 0 lands
                xt0 = px.tile([128, 32, 256], BF, tag="xt")
                nc.sync.dma_start(xt0[:], xT_r[:, :, 0:256])
                rcc0 = px.tile([128, 2, 5, 128], BF, tag="rcc")
                nc.sync.dma_start(rcc0[:], rcc_r[:, 0:2, :, :])
                rss0 = px.tile([128, 2, 5, 128], BF, tag="rss")
                nc.sync.dma_start(rss0[:], rss_r[:, 0:2, :, :])
                for jc in range(4):
                    nc.sync.dma_start(wq_sb[:, 8 * jc:8 * (jc + 1), :],
                                      wq_r[:, 8 * jc:8 * (jc + 1), :])
                    nc.sync.dma_start(wkv_sb[:, 8 * jc:8 * (jc + 1), :],
                                      wkv_r[:, 8 * jc:8 * (jc + 1), :])

                for i2 in range(NT // 2):
                    if i2 == 0:
                        xt, rcc, rss = xt0, rcc0, rss0
                    else:
                        xt = px.tile([128, 32, 256], BF, tag="xt")
                        nc.sync.dma_start(xt[:], xT_r[:, :, 256 * i2:256 * (i2 + 1)])
                        rcc = px.tile([128, 2, 5, 128], BF, tag="rcc")
                        nc.sync.dma_start(rcc[:], rcc_r[:, 2 * i2:2 * i2 + 2, :, :])
                        rss = px.tile([128, 2, 5, 128], BF, tag="rss")
                        nc.sync.dma_start(rss[:], rss_r[:, 2 * i2:2 * i2 + 2, :, :])
                    for ii in range(2):
                        i = 2 * i2 + ii
                        psq = ps1.tile([128, QH, HD], F32, tag="psq")
                        pskv = ps1.tile([128, 2, HD], F32, tag="pskv")
                        for j in range(32):
                            xs = xt[:, j, 128 * ii:128 * (ii + 1)]
                            nc.tensor.matmul(psq[:], xs, wq_sb[:, j, :],
                                             start=(j == 0), stop=(j == 31))
                            nc.tensor.matmul(pskv[:], xs, wkv_sb[:, j, :],
                                             start=(j == 0), stop=(j == 31))
                        # v to resident (DVE, casts to bf16)
                        nc.vector.tensor_copy(out=v_sb[:, i, :], in_=pskv[:, 1, :])
                        # combined q(4 heads)+k tile [128, 5, 128] in bf16
                        qkc = p1.tile([128, 5, 128], BF, tag="qkc")
                        nc.scalar.copy(qkc[:, 0:4, :], psq[:])
                        nc.scalar.copy(qkc[:, 4, :], pskv[:, 0, :])
                        # half-swapped copy for rope
                        qksw = p1.tile([128, 5, 128], BF, tag="qksw")
                        nc.scalar.copy(qksw[:, :, 0:64], qkc[:, :, 64:128])
                        nc.scalar.copy(qksw[:, :, 64:128], qkc[:, :, 0:64])
                        # rms stats
                        sq = p1.tile([128, 5, 128], BF, tag="sq")
                        nc.vector.tensor_tensor(sq[:], qkc[:], qkc[:], MUL)
                        ssq = p1.tile([128, 5, 1], F32, tag="ssq")
                        nc.vector.reduce_sum(ssq[:], sq[:], axis=X_AX)
                        var = p1.tile([128, 5, 1], F32, tag="var")
                        nc.vector.tensor_scalar(var[:], ssq[:], 1.0 / HD, EPS,
                                                MUL, ADD)
                        rms = p1.tile([128, 5, 1], F32, tag="rms")
                        nc.scalar.activation(rms[:], var[:], AF.Sqrt)
                        inv = p1.tile([128, 5, 1], F32, tag="inv")
                        nc.vector.reciprocal(inv[:], rms[:])
                        # rope (tables carry norm-w; q tables also softmax scale)
                        t1 = p1.tile([128, 5, 128], BF, tag="t1")
                        nc.vector.tensor_tensor(t1[:], qkc[:], rcc[:, ii], MUL)
                        t2 = p1.tile([128, 5, 128], BF, tag="t2")
                        nc.vector.tensor_tensor(t2[:], qksw[:], rss[:, ii], MUL)
                        qr = p1.tile([128, 5, 128], BF, tag="qr")
                        nc.vector.tensor_tensor(qr[:], t1[:], t2[:], ADD)
                        qn = p1.tile([128, 5, 128], BF, tag="qn")
                        for c5 in range(5):
                            nc.vector.tensor_scalar_mul(
                                qn[:, c5, :], qr[:, c5, :], inv[:, c5, :])
                        # transpose into resident [hd, tok] layouts
                        with nc.allow_low_precision(reason="pure transpose"):
                            for h in range(QH):
                                pt = pst.tile([128, 128], BF, tag="pt")
                                nc.tensor.transpose(pt[:], qn[:, h, :], ident_sb[:])
                                nc.vector.tensor_copy(
                                    out=qT_sb[:, h, 128 * i:128 * (i + 1)], in_=pt[:])
                            pt = pst.tile([128, 128], BF, tag="pt")
                            nc.tensor.transpose(pt[:], qn[:, 4, :], ident_sb[:])
                            nc.vector.tensor_copy(
                                out=kT_sb[:, 128 * i:128 * (i + 1)], in_=pt[:])

            # ---------------- phase 2: attention + per-head a2a -------------
            with (
                tc.tile_pool(name="p2", bufs=4) as p2,
                tc.tile_pool(name="ps2", bufs=3, space="PSUM") as ps2,
                tc.tile_pool(name="pso", bufs=2, space="PSUM") as pso,
                tc.tile_pool(name="psb2", bufs=1, space="PSUM") as psbp,
            ):
                for h in range(QH):
                    for b in range(B):
                        for qb in range(4):
                            q0 = b * L + QB * qb
                            nkt = 4 * qb + 4
                            pso_o = pso.tile([128, QB], F32, tag="o")
                            pso_s = pso.tile([1, QB], F32, tag="s")
                            acc = p2.tile([128, QB], F32, tag="acc")
                            # 2-deep software pipeline: scores(kt+1, kt+2)
                            # issue on the tensor queue before o(kt), so the
                            # PE never waits on exp; the softmax-denominator
                            # sum accumulates on the idle gpsimd engine
                            pend = []
                            for kt in range(nkt):
                                kc_ = b * L + 128 * kt
                                pss = ps2.tile([128, QB], F32, tag="pss")
                                nc.tensor.matmul(pss[:], kT_sb[:, kc_:kc_ + 128],
                                                 qT_sb[:, h, q0:q0 + QB],
                                                 start=True, stop=True)
                                pT = p2.tile([128, QB], BF, tag="pT")
                                nc.scalar.activation(pT[:], pss[:], AF.Exp)
                                t = kt - 4 * qb
                                if t >= 0:
                                    off = 384 - 128 * t
                                    nc.vector.tensor_tensor(
                                        pT[:], pT[:], pat_sb[:, off:off + QB], MUL)
                                if kt == 0:
                                    nc.gpsimd.tensor_copy(out=acc[:], in_=pT[:])
                                else:
                                    nc.gpsimd.tensor_tensor(acc[:], acc[:], pT[:], ADD)
                                pend.append((kt, pT))
                                if len(pend) > 2:
                                    pkt, ppT = pend.pop(0)
                                    nc.tensor.matmul(pso_o[:],
                                                     v_sb[:, b * 16 + pkt, :], ppT[:],
                                                     start=(pkt == 0),
                                                     stop=(pkt == nkt - 1))
                            for pkt, ppT in pend:
                                nc.tensor.matmul(pso_o[:], v_sb[:, b * 16 + pkt, :],
                                                 ppT[:], start=(pkt == 0),
                                                 stop=(pkt == nkt - 1))
                            acc16 = p2.tile([128, QB], BF, tag="acc16")
                            nc.vector.tensor_copy(out=acc16[:], in_=acc[:])
                            nc.tensor.matmul(pso_s[:], onesc_sb[:], acc16[:],
                                             start=True, stop=True)
                            rec = p2.tile([1, QB], F32, tag="rec")
                            nc.vector.reciprocal_approx_fast(rec[:], pso_s[:])
                            rec16 = p2.tile([1, QB], BF, tag="rec16")
                            nc.vector.tensor_copy(out=rec16[:], in_=rec[:])
                            psb = psbp.tile([128, QB], F32, tag="psb")
                            nc.tensor.matmul(psb[:], onesr_sb[:], rec16[:],
                                             start=True, stop=True)
                            bcs = p2.tile([128, QB], BF, tag="bcs")
                            nc.vector.tensor_copy(out=bcs[:], in_=psb[:])
                            attn = p2.tile([128, QB], BF, tag="attn")
                            nc.vector.tensor_tensor(attn[:], pso_o[:], bcs[:], MUL)
                            j = 4 * b + qb
                            nc.sync.dma_start(a2a_in[h][j, :, :], attn[:])
                    nc.gpsimd.collective_compute(
                        "AllToAll", mybir.AluOpType.bypass,
                        replica_groups=[list(range(NC_))],
                        ins=[a2a_in[h].opt()], outs=[a2a_out[h].opt()])

            # ---------------- phase 4: output projection --------------------
            with (
                tc.tile_pool(name="p4a", bufs=1) as p4a,
                tc.tile_pool(name="p4w", bufs=3) as p4w,
                tc.tile_pool(name="p4o", bufs=2) as p4o,
                tc.tile_pool(name="ps4", bufs=2, space="PSUM") as ps4,
            ):
                at = []
                for h in range(QH):
                    ath = p4a.tile([128, NC_, QB], BF, name=f"at{h}")
                    nc.sync.dma_start(
                        ath[:], a2a_out[h][:].rearrange("s p t -> p s t"))
                    at.append(ath)
                def emit_chunks(oc, pso_list, hs):
                    for h4 in hs:
                        wt = p4w.tile([128, 8, 512], BF, tag="wt")
                        nc.sync.dma_start(
                            wt[:], wo_r[:, 8 * h4:8 * (h4 + 1), 512 * oc:512 * (oc + 1)])
                        for s in range(8):
                            k = 8 * h4 + s
                            for tt in range(4):
                                nc.tensor.matmul(
                                    pso_list[tt][:],
                                    at[h4][:, s, 128 * tt:128 * (tt + 1)],
                                    wt[:, s, :], start=(k == 0), stop=(k == 31))

                def flush(oc, pso_list):
                    for tt in range(4):
                        ob = p4o.tile([128, 512], F32, tag="ob")
                        nc.vector.tensor_copy(out=ob[:], in_=pso_list[tt][:])
                        nc.sync.dma_start(
                            out[128 * tt:128 * (tt + 1), 512 * oc:512 * (oc + 1)], ob[:])

                # skewed: defer each oc's h3 chunk until after the next oc's
                # h0-h2, so the wait for the last a2a overlaps real work
                pend4 = None
                for oc in range(8):
                    pso_list = [ps4.tile([128, 512], F32, tag=f"po{tt}", name=f"po{tt}")
                                for tt in range(4)]
                    emit_chunks(oc, pso_list, [0, 1, 2])
                    if pend4 is not None:
                        emit_chunks(pend4[0], pend4[1], [3])
                        flush(*pend4)
                    pend4 = (oc, pso_list)
                emit_chunks(pend4[0], pend4[1], [3])
                flush(*pend4)

    nc.compile()
    return nc


def _prep(inputs):
    x = np.asarray(inputs["x"], np.float32)
    wq = np.asarray(inputs["wq"], np.float32)
    wk = np.asarray(inputs["wk"], np.float32)
    wv = np.asarray(inputs["wv"], np.float32)
    wo = np.asarray(inputs["wo"], np.float32)
    qw = np.asarray(inputs["q_norm_w"], np.float32)
    kw = np.asarray(inputs["k_norm_w"], np.float32)

    xT = np.ascontiguousarray(x.reshape(T, D).T).astype(BF16NP)

    half = HD // 2
    inv_freq = 1.0 / (ROPE_BASE ** (np.arange(half, dtype=np.float32) / half))
    pos = np.arange(L, dtype=np.float32)
    ang = pos[:, None] * inv_freq[None, :]
    cos = np.cos(ang).astype(np.float32)
    sin = np.sin(ang).astype(np.float32)
    scale = np.float32(HD ** -0.5)

    def rope_tabs(w, s):
        # out[j] = qk[j]*cc[j] + qksw[j]*ss[j]   (qksw = half-swapped qk)
        cc = np.concatenate([cos * w[None, 0:half], cos * w[None, half:HD]], axis=1) * s
        ss = np.concatenate([-sin * w[None, half:HD], sin * w[None, 0:half]], axis=1) * s
        return cc, ss  # each [L, HD]

    ccq, ssq_ = rope_tabs(qw, scale)
    cck, ssk = rope_tabs(kw, np.float32(1.0))
    ropecc = np.concatenate([ccq, ccq, ccq, ccq, cck], axis=1)   # [L, 640]
    ropess = np.concatenate([ssq_, ssq_, ssq_, ssq_, ssk], axis=1)
    ropecc = np.ascontiguousarray(np.tile(ropecc, (B, 1))).astype(BF16NP)
    ropess = np.ascontiguousarray(np.tile(ropess, (B, 1))).astype(BF16NP)

    kk = np.arange(128)[:, None]
    cc_ = np.arange(896)[None, :]
    pat = (kk <= cc_ - 384).astype(BF16NP)
    ident = np.eye(128, dtype=BF16NP)
    ones_c = np.ones((128, 1), BF16NP)
    ones_r = np.ones((1, 128), BF16NP)

    # wo rows permuted to (h_local, src_core, hd) chunk order
    perm = np.concatenate([
        np.arange(128) + (s * QH + h4) * 128
        for h4 in range(QH) for s in range(NC_)])
    wo_p = np.ascontiguousarray(wo[perm, :]).astype(BF16NP)

    in_maps = []
    for c in range(NC_):
        in_maps.append({
            "xT": xT,
            "wq": np.ascontiguousarray(wq[:, 512 * c:512 * (c + 1)]).astype(BF16NP),
            "wkv": np.ascontiguousarray(np.concatenate(
                [wk[:, HD * c:HD * (c + 1)], wv[:, HD * c:HD * (c + 1)]],
                axis=1)).astype(BF16NP),
            "wo": wo_p,
            "ropecc": ropecc,
            "ropess": ropess,
            "pat": pat,
            "ident": ident,
            "ones_c": ones_c,
            "ones_r": ones_r,
        })
    return in_maps


def kernel(**inputs) -> np.ndarray:
    if "nc" not in _CACHE:
        _CACHE["nc"] = _build()
    nc = _CACHE["nc"]
    in_maps = _prep(inputs)
    res = run_bass_kernel_spmd(nc, in_maps, list(range(NC_)))
    chunks = [res.results[c]["out"] for c in range(NC_)]
    return np.concatenate(chunks, axis=0).reshape(B, L, D)


# revision 12
# speedup vs baseline: 1.1113x; 1.1113x over previous
"""GQA attention layer (B=2, L=2048, D=4096, H=32, KH=8, HD=128) on 8 TRN2 cores.

Sharding: tensor-parallel over KV heads (1 kv head + 4 q heads per core).
All matmul operands in bf16 (fp32 streams at half rate on TRN2 PE and gets no
FWL weight loads). Per core: QKV projection (x^T-tile stationary), fused
per-head RMSNorm + RoPE via host-precomputed cc/ss tables (norm weight and
softmax scale folded in), PE transposes into resident SBUF qT/kT, flash-style
attention in S^T layout with causal block skipping, per-head AllToAll (4
collectives, pipelined against attention + output projection), then the output
projection for this core's 512-token slice. Host assembles the 8 token slices.
"""
import numpy as np
import ml_dtypes

import concourse.bass as bass
import concourse.mybir as mybir
import concourse.tile as tile
from concourse import bacc
from concourse.bass_utils import run_bass_kernel_spmd

F32 = mybir.dt.float32
F32R = mybir.dt.float32r
BF = mybir.dt.bfloat16
BF16NP = ml_dtypes.bfloat16
AF = mybir.ActivationFunctionType
MUL = mybir.AluOpType.mult
ADD = mybir.AluOpType.add
X_AX = mybir.AxisListType.X

B, L, D = 2, 2048, 4096
H, KH, HD = 32, 8, 128
T = B * L              # 4096 tokens
NC_ = 8                # cores
QH = H // NC_          # 4 q heads per core
QB = 512               # q block
NT = T // 128          # 32 token tiles
EPS = 1e-5
ROPE_BASE = 1000000.0

_CACHE = {}


def _build():
    nc = bacc.Bacc("TRN2", target_bir_lowering=False, debug=False, num_devices=NC_)

    xT = nc.dram_tensor("xT", [D, T], BF, kind="ExternalInput").ap()
    wq = nc.dram_tensor("wq", [D, QH * HD], BF, kind="ExternalInput").ap()
    wkv = nc.dram_tensor("wkv", [D, 2 * HD], BF, kind="ExternalInput").ap()
    wo = nc.dram_tensor("wo", [D, D], BF, kind="ExternalInput").ap()
    ropecc = nc.dram_tensor("ropecc", [T, 5 * HD], BF, kind="ExternalInput").ap()
    ropess = nc.dram_tensor("ropess", [T, 5 * HD], BF, kind="ExternalInput").ap()
    pat = nc.dram_tensor("pat", [128, 896], BF, kind="ExternalInput").ap()
    ident = nc.dram_tensor("ident", [128, 128], BF, kind="ExternalInput").ap()
    ones_c = nc.dram_tensor("ones_c", [128, 1], BF, kind="ExternalInput").ap()
    ones_r = nc.dram_tensor("ones_r", [1, 128], BF, kind="ExternalInput").ap()
    out = nc.dram_tensor("out", [T // NC_, D], F32, kind="ExternalOutput").ap()

    xT_r = xT.rearrange("(o p) t -> p o t", p=128)         # [128, 32, T]
    wq_r = wq.rearrange("(o p) n -> p o n", p=128)         # [128, 32, 512]
    wkv_r = wkv.rearrange("(o p) n -> p o n", p=128)       # [128, 32, 256]
    wo_r = wo.rearrange("(o p) n -> p o n", p=128)         # [128, 32, 4096]
    rcc_r = ropecc.rearrange("(n p) (c j) -> p n c j", p=128, j=128)  # [128,32,5,128]
    rss_r = ropess.rearrange("(n p) (c j) -> p n c j", p=128, j=128)

    with tile.TileContext(nc) as tc:
        with (
            tc.tile_pool(name="const", bufs=1) as cp,
            tc.tile_pool(name="dram", bufs=1, space="DRAM") as dramp,
            tc.tile_pool(name="res", bufs=1) as resp,
        ):
            ident_sb = cp.tile([128, 128], BF)
            nc.sync.dma_start(ident_sb[:], ident)
            pat_sb = cp.tile([128, 896], BF)
            nc.sync.dma_start(pat_sb[:], pat)
            onesc_sb = cp.tile([128, 1], BF)
            nc.sync.dma_start(onesc_sb[:], ones_c)
            onesr_sb = cp.tile([1, 128], BF)
            nc.sync.dma_start(onesr_sb[:], ones_r)

            kT_sb = resp.tile([128, T], BF)                # [hd, tok]
            v_sb = resp.tile([128, NT, HD], BF)            # [tok%128, tile, hd]
            qT_sb = resp.tile([128, QH, T], BF)            # [hd, head, tok]

            a2a_in = [dramp.tile([NC_, HD, QB], BF, name=f"a2a_in{h}")
                      for h in range(QH)]
            a2a_out = [dramp.tile([NC_, HD, QB], BF, name=f"a2a_out{h}")
                       for h in range(QH)]

            # ---------------- phase 1: projections + norm + rope ----------
            with (
                tc.tile_pool(name="wts", bufs=1) as wp,
                tc.tile_pool(name="px", bufs=2) as px,
                tc.tile_pool(name="p1", bufs=2) as p1,
                tc.tile_pool(name="ps1", bufs=2, space="PSUM") as ps1,
                tc.tile_pool(name="pst", bufs=2, space="PSUM") as pst,
            ):
                wq_sb = wp.tile([128, 32, QH * HD], BF)
                wkv_sb = wp.tile([128, 32, 2 * HD], BF)
                # first x/table tiles ahead of the big weight DMAs so the
                # tensor engine starts as soon as weight chunk 0 lands
                xt0 = px.tile([128, 32, 256], BF, tag="xt")
                nc.sync.dma_start(xt0[:], xT_r[:, :, 0:256])
                rcc0 = px.tile([128, 2, 5, 128], BF, tag="rcc")
                nc.sync.dma_start(rcc0[:], rcc_r[:, 0:2, :, :])
                rss0 = px.tile([128, 2, 5, 128], BF, tag="rss")
                nc.sync.dma_start(rss0[:], rss_r[:, 0:2, :, :])
                for jc in range(4):
                    nc.sync.dma_start(wq_sb[:, 8 * jc:8 * (jc + 1), :],
                                      wq_r[:, 8 * jc:8 * (jc + 1), :])
                    nc.sync.dma_start(wkv_sb[:, 8 * jc:8 * (jc + 1), :],
                                      wkv_r[:, 8 * jc:8 * (jc + 1), :])

                for i2 in range(NT // 2):
                    if i2 == 0:
                        xt, rcc, rss = xt0, rcc0, rss0
                    else:
                        xt = px.tile([128, 32, 256], BF, tag="xt")
                        nc.sync.dma_start(xt[:], xT_r[:, :, 256 * i2:256 * (i2 + 1)])
                        rcc = px.tile([128, 2, 5, 128], BF, tag="rcc")
                        nc.sync.dma_start(rcc[:], rcc_r[:, 2 * i2:2 * i2 + 2, :, :])
                        rss = px.tile([128, 2, 5, 128], BF, tag="rss")
                        nc.sync.dma_start(rss[:], rss_r[:, 2 * i2:2 * i2 + 2, :, :])
                    for ii in range(2):
                        i = 2 * i2 + ii
                        psq = ps1.tile([128, QH, HD], F32, tag="psq")
                        pskv = ps1.tile([128, 2, HD], F32, tag="pskv")
                        for j in range(32):
                            xs = xt[:, j, 128 * ii:128 * (ii + 1)]
                            nc.tensor.matmul(psq[:], xs, wq_sb[:, j, :],
                                             start=(j == 0), stop=(j == 31))
                            nc.tensor.matmul(pskv[:], xs, wkv_sb[:, j, :],
                                             start=(j == 0), stop=(j == 31))
                        # v to resident (DVE, casts to bf16)
                        nc.vector.tensor_copy(out=v_sb[:, i, :], in_=pskv[:, 1, :])
                        # combined q(4 heads)+k tile [128, 5, 128] in bf16
                        qkc = p1.tile([128, 5, 128], BF, tag="qkc")
                        nc.scalar.copy(qkc[:, 0:4, :], psq[:])
                        nc.scalar.copy(qkc[:, 4, :], pskv[:, 0, :])
                        # half-swapped copy for rope
                        qksw = p1.tile([128, 5, 128], BF, tag="qksw")
                        nc.scalar.copy(qksw[:, :, 0:64], qkc[:, :, 64:128])
                        nc.scalar.copy(qksw[:, :, 64:128], qkc[:, :, 0:64])
                        # rms stats
                        sq = p1.tile([128, 5, 128], BF, tag="sq")
                        nc.vector.tensor_tensor(sq[:], qkc[:], qkc[:], MUL)
                        ssq = p1.tile([128, 5, 1], F32, tag="ssq")
                        nc.vector.reduce_sum(ssq[:], sq[:], axis=X_AX)
                        var = p1.tile([128, 5, 1], F32, tag="var")
                        nc.vector.tensor_scalar(var[:], ssq[:], 1.0 / HD, EPS,
                                                MUL, ADD)
                        rms = p1.tile([128, 5, 1], F32, tag="rms")
                        nc.scalar.activation(rms[:], var[:], AF.Sqrt)
                        inv = p1.tile([128, 5, 1], F32, tag="inv")
                        nc.vector.reciprocal(inv[:], rms[:])
                        # rope (tables carry norm-w; q tables also softmax scale)
                        t1 = p1.tile([128, 5, 128], BF, tag="t1")
                        nc.vector.tensor_tensor(t1[:], qkc[:], rcc[:, ii], MUL)
                        t2 = p1.tile([128, 5, 128], BF, tag="t2")
                        nc.vector.tensor_tensor(t2[:], qksw[:], rss[:, ii], MUL)
                        qr = p1.tile([128, 5, 128], BF, tag="qr")
                        nc.vector.tensor_tensor(qr[:], t1[:], t2[:], ADD)
                        qn = p1.tile([128, 5, 128], BF, tag="qn")
                        for c5 in range(5):
                            nc.vector.tensor_scalar_mul(
                                qn[:, c5, :], qr[:, c5, :], inv[:, c5, :])
                        # transpose into resident [hd, tok] layouts
                        with nc.allow_low_precision(reason="pure transpose"):
                            for h in range(QH):
                                pt = pst.tile([128, 128], BF, tag="pt")
                                nc.tensor.transpose(pt[:], qn[:, h, :], ident_sb[:])
                                nc.vector.tensor_copy(
                                    out=qT_sb[:, h, 128 * i:128 * (i + 1)], in_=pt[:])
                            pt = pst.tile([128, 128], BF, tag="pt")
                            nc.tensor.transpose(pt[:], qn[:, 4, :], ident_sb[:])
                            nc.vector.tensor_copy(
                                out=kT_sb[:, 128 * i:128 * (i + 1)], in_=pt[:])

            # ---------------- phase 2: attention + per-head a2a -------------
            with (
                tc.tile_pool(name="p2", bufs=4) as p2,
                tc.tile_pool(name="ps2", bufs=3, space="PSUM") as ps2,
                tc.tile_pool(name="pso", bufs=2, space="PSUM") as pso,
                tc.tile_pool(name="psb2", bufs=1, space="PSUM") as psbp,
            ):
                for h in range(QH):
                    for b in range(B):
                        for qb in range(4):
                            q0 = b * L + QB * qb
                            nkt = 4 * qb + 4
                            pso_o = pso.tile([128, QB], F32, tag="o")
                            pso_s = pso.tile([1, QB], F32, tag="s")
                            # 2-deep software pipeline: scores(kt+1, kt+2)
                            # issue on the tensor queue before s/o(kt), so
                            # the PE never waits on exp(kt)
                            pend = []

                            def flush_so(pkt, ppT):
                                nc.tensor.matmul(pso_s[:], onesc_sb[:], ppT[:],
                                                 start=(pkt == 0),
                                                 stop=(pkt == nkt - 1))
                                nc.tensor.matmul(pso_o[:], v_sb[:, b * 16 + pkt, :],
                                                 ppT[:], start=(pkt == 0),
                                                 stop=(pkt == nkt - 1))

                            for kt in range(nkt):
                                kc_ = b * L + 128 * kt
                                pss = ps2.tile([128, QB], F32, tag="pss")
                                nc.tensor.matmul(pss[:], kT_sb[:, kc_:kc_ + 128],
                                                 qT_sb[:, h, q0:q0 + QB],
                                                 start=True, stop=True)
                                pT = p2.tile([128, QB], BF, tag="pT")
                                nc.scalar.activation(pT[:], pss[:], AF.Exp)
                                t = kt - 4 * qb
                                if t >= 0:
                                    off = 384 - 128 * t
                                    nc.vector.tensor_tensor(
                                        pT[:], pT[:], pat_sb[:, off:off + QB], MUL)
                                pend.append((kt, pT))
                                if len(pend) > 2:
                                    flush_so(*pend.pop(0))
                            for pe_ in pend:
                                flush_so(*pe_)
                            rec = p2.tile([1, QB], F32, tag="rec")
                            nc.vector.reciprocal_approx_fast(rec[:], pso_s[:])
                            rec16 = p2.tile([1, QB], BF, tag="rec16")
                            nc.vector.tensor_copy(out=rec16[:], in_=rec[:])
                            psb = psbp.tile([128, QB], F32, tag="psb")
                            nc.tensor.matmul(psb[:], onesr_sb[:], rec16[:],
                                             start=True, stop=True)
                            bcs = p2.tile([128, QB], BF, tag="bcs")
                            nc.vector.tensor_copy(out=bcs[:], in_=psb[:])
                            attn = p2.tile([128, QB], BF, tag="attn")
                            nc.vector.tensor_tensor(attn[:], pso_o[:], bcs[:], MUL)
                            j = 4 * b + qb
                            nc.sync.dma_start(a2a_in[h][j, :, :], attn[:])
                    nc.gpsimd.collective_compute(
                        "AllToAll", mybir.AluOpType.bypass,
                        replica_groups=[list(range(NC_))],
                        ins=[a2a_in[h].opt()], outs=[a2a_out[h].opt()])

            # ---------------- phase 4: output projection --------------------
            with (
                tc.tile_pool(name="p4a", bufs=1) as p4a,
                tc.tile_pool(name="p4w", bufs=3) as p4w,
                tc.tile_pool(name="p4o", bufs=2) as p4o,
                tc.tile_pool(name="ps4", bufs=2, space="PSUM") as ps4,
            ):
                at = []
                for h in range(QH):
                    ath = p4a.tile([128, NC_, QB], BF, name=f"at{h}")
                    nc.sync.dma_start(
                        ath[:], a2a_out[h][:].rearrange("s p t -> p s t"))
                    at.append(ath)
                def emit_chunks(oc, pso_list, hs):
                    for h4 in hs:
                        wt = p4w.tile([128, 8, 512], BF, tag="wt")
                        nc.sync.dma_start(
                            wt[:], wo_r[:, 8 * h4:8 * (h4 + 1), 512 * oc:512 * (oc + 1)])
                        for s in range(8):
                            k = 8 * h4 + s
                            for tt in range(4):
                                nc.tensor.matmul(
                                    pso_list[tt][:],
                                    at[h4][:, s, 128 * tt:128 * (tt + 1)],
                                    wt[:, s, :], start=(k == 0), stop=(k == 31))

                def flush(oc, pso_list):
                    for tt in range(4):
                        ob = p4o.tile([128, 512], F32, tag="ob")
                        nc.vector.tensor_copy(out=ob[:], in_=pso_list[tt][:])
                        nc.sync.dma_start(
                            out[128 * tt:128 * (tt + 1), 512 * oc:512 * (oc + 1)], ob[:])

                # skewed: defer each oc's h3 chunk until after the next oc's
                # h0-h2, so the wait for the last a2a overlaps real work
                pend4 = None
                for oc in range(8):
                    pso_list = [ps4.tile([128, 512], F32, tag=f"po{tt}", name=f"po{tt}")
                                for tt in range(4)]
                    emit_chunks(oc, pso_list, [0, 1, 2])
                    if pend4 is not None:
                        emit_chunks(pend4[0], pend4[1], [3])
                        flush(*pend4)
                    pend4 = (oc, pso_list)
                emit_chunks(pend4[0], pend4[1], [3])
                flush(*pend4)

    nc.compile()
    return nc


def _prep(inputs):
    x = np.asarray(inputs["x"], np.float32)
    wq = np.asarray(inputs["wq"], np.float32)
    wk = np.asarray(inputs["wk"], np.float32)
    wv = np.asarray(inputs["wv"], np.float32)
    wo = np.asarray(inputs["wo"], np.float32)
    qw = np.asarray(inputs["q_norm_w"], np.float32)
    kw = np.asarray(inputs["k_norm_w"], np.float32)

    xT = np.ascontiguousarray(x.reshape(T, D).T).astype(BF16NP)

    half = HD // 2
    inv_freq = 1.0 / (ROPE_BASE ** (np.arange(half, dtype=np.float32) / half))
    pos = np.arange(L, dtype=np.float32)
    ang = pos[:, None] * inv_freq[None, :]
    cos = np.cos(ang).astype(np.float32)
    sin = np.sin(ang).astype(np.float32)
    scale = np.float32(HD ** -0.5)

    def rope_tabs(w, s):
        # out[j] = qk[j]*cc[j] + qksw[j]*ss[j]   (qksw = half-swapped qk)
        cc = np.concatenate([cos * w[None, 0:half], cos * w[None, half:HD]], axis=1) * s
        ss = np.concatenate([-sin * w[None, half:HD], sin * w[None, 0:half]], axis=1) * s
        return cc, ss  # each [L, HD]

    ccq, ssq_ = rope_tabs(qw, scale)
    cck, ssk = rope_tabs(kw, np.float32(1.0))
    ropecc = np.concatenate([ccq, ccq, ccq, ccq, cck], axis=1)   # [L, 640]
    ropess = np.concatenate([ssq_, ssq_, ssq_, ssq_, ssk], axis=1)
    ropecc = np.ascontiguousarray(np.tile(ropecc, (B, 1))).astype(BF16NP)
    ropess = np.ascontiguousarray(np.tile(ropess, (B, 1))).astype(BF16NP)

    kk = np.arange(128)[:, None]
    cc_ = np.arange(896)[None, :]
    pat = (kk <= cc_ - 384).astype(BF16NP)
    ident = np.eye(128, dtype=BF16NP)
    ones_c = np.ones((128, 1), BF16NP)
    ones_r = np.ones((1, 128), BF16NP)

    # wo rows permuted to (h_local, src_core, hd) chunk order
    perm = np.concatenate([
        np.arange(128) + (s * QH + h4) * 128
        for h4 in range(QH) for s in range(NC_)])
    wo_p = np.ascontiguousarray(wo[perm, :]).astype(BF16NP)

    in_maps = []
    for c in range(NC_):
        in_maps.append({
            "xT": xT,
            "wq": np.ascontiguousarray(wq[:, 512 * c:512 * (c + 1)]).astype(BF16NP),
            "wkv": np.ascontiguousarray(np.concatenate(
                [wk[:, HD * c:HD * (c + 1)], wv[:, HD * c:HD * (c + 1)]],
                axis=1)).astype(BF16NP),
            "wo": wo_p,
            "ropecc": ropecc,
            "ropess": ropess,
            "pat": pat,
            "ident": ident,
            "ones_c": ones_c,
            "ones_r": ones_r,
        })
    return in_maps


def kernel(**inputs) -> np.ndarray:
    if "nc" not in _CACHE:
        _CACHE["nc"] = _build()
    nc = _CACHE["nc"]
    in_maps = _prep(inputs)
    res = run_bass_kernel_spmd(nc, in_maps, list(range(NC_)))
    chunks = [res.results[c]["out"] for c in range(NC_)]
    return np.concatenate(chunks, axis=0).reshape(B, L, D)


# revision 16
# speedup vs baseline: 1.1603x; 1.0441x over previous
"""GQA attention layer (B=2, L=2048, D=4096, H=32, KH=8, HD=128) on 8 TRN2 cores.

Sharding: tensor-parallel over KV heads (1 kv head + 4 q heads per core).
All matmul operands in bf16 (fp32 streams at half rate on TRN2 PE and gets no
FWL weight loads). Per core: QKV projection (x^T-tile stationary), fused
per-head RMSNorm + RoPE via host-precomputed cc/ss tables (norm weight and
softmax scale folded in), PE transposes into resident SBUF qT/kT, flash-style
attention in S^T layout with causal block skipping, per-head AllToAll (4
collectives, pipelined against attention + output projection), then the output
projection for this core's 512-token slice. Host assembles the 8 token slices.
"""
import numpy as np
import ml_dtypes

import concourse.bass as bass
import concourse.mybir as mybir
import concourse.tile as tile
from concourse import bacc
from concourse.bass_utils import run_bass_kernel_spmd

F32 = mybir.dt.float32
F32R = mybir.dt.float32r
BF = mybir.dt.bfloat16
BF16NP = ml_dtypes.bfloat16
AF = mybir.ActivationFunctionType
MUL = mybir.AluOpType.mult
ADD = mybir.AluOpType.add
X_AX = mybir.AxisListType.X

B, L, D = 2, 2048, 4096
H, KH, HD = 32, 8, 128
T = B * L              # 4096 tokens
NC_ = 8                # cores
QH = H // NC_          # 4 q heads per core
QB = 512               # q block
NT = T // 128          # 32 token tiles
EPS = 1e-5
ROPE_BASE = 1000000.0

_CACHE = {}


def _build():
    nc = bacc.Bacc("TRN2", target_bir_lowering=False, debug=False, num_devices=NC_)

    xT = nc.dram_tensor("xT", [D, T], BF, kind="ExternalInput").ap()
    wq = nc.dram_tensor("wq", [D, QH * HD], BF, kind="ExternalInput").ap()
    wkv = nc.dram_tensor("wkv", [D, 2 * HD], BF, kind="ExternalInput").ap()
    wo = nc.dram_tensor("wo", [D, D], BF, kind="ExternalInput").ap()
    ropecc = nc.dram_tensor("ropecc", [T, 5 * HD], BF, kind="ExternalInput").ap()
    ropess = nc.dram_tensor("ropess", [T, 5 * HD], BF, kind="ExternalInput").ap()
    pat = nc.dram_tensor("pat", [128, 896], BF, kind="ExternalInput").ap()
    ident = nc.dram_tensor("ident", [128, 128], BF, kind="ExternalInput").ap()
    ones_c = nc.dram_tensor("ones_c", [128, 1], BF, kind="ExternalInput").ap()
    ones_r = nc.dram_tensor("ones_r", [1, 128], BF, kind="ExternalInput").ap()
    out = nc.dram_tensor("out", [T // NC_, D], F32, kind="ExternalOutput").ap()

    xT_r = xT.rearrange("(o p) t -> p o t", p=128)         # [128, 32, T]
    wq_r = wq.rearrange("(o p) n -> p o n", p=128)         # [128, 32, 512]
    wkv_r = wkv.rearrange("(o p) n -> p o n", p=128)       # [128, 32, 256]
    wo_r = wo.rearrange("(o p) n -> p o n", p=128)         # [128, 32, 4096]
    rcc_r = ropecc.rearrange("(n p) (c j) -> p n c j", p=128, j=128)  # [128,32,5,128]
    rss_r = ropess.rearrange("(n p) (c j) -> p n c j", p=128, j=128)

    with tile.TileContext(nc) as tc:
        with (
            tc.tile_pool(name="const", bufs=1) as cp,
            tc.tile_pool(name="dram", bufs=1, space="DRAM") as dramp,
            tc.tile_pool(name="res", bufs=1) as resp,
        ):
            ident_sb = cp.tile([128, 128], BF)
            nc.sync.dma_start(ident_sb[:], ident)
            pat_sb = cp.tile([128, 896], BF)
            nc.sync.dma_start(pat_sb[:], pat)
            onesc_sb = cp.tile([128, 1], BF)
            nc.sync.dma_start(onesc_sb[:], ones_c)
            onesr_sb = cp.tile([1, 128], BF)
            nc.sync.dma_start(onesr_sb[:], ones_r)

            kT_sb = resp.tile([128, T], BF)                # [hd, tok]
            v_sb = resp.tile([128, NT, HD], BF)            # [tok%128, tile, hd]
            qT_sb = resp.tile([128, QH, T], BF)            # [hd, head, tok]

            a2a_in = [dramp.tile([NC_, HD, QB], BF, name=f"a2a_in{h}")
                      for h in range(QH)]
            a2a_out = [dramp.tile([NC_, HD, QB], BF, name=f"a2a_out{h}")
                       for h in range(QH)]

            # ---------------- phase 1: projections + norm + rope ----------
            with (
                tc.tile_pool(name="wts", bufs=1) as wp,
                tc.tile_pool(name="px", bufs=2) as px,
                tc.tile_pool(name="p1", bufs=2) as p1,
                tc.tile_pool(name="ps1", bufs=3, space="PSUM") as ps1,
                tc.tile_pool(name="pst", bufs=2, space="PSUM") as pst,
            ):
                wq_sb = wp.tile([128, 32, QH * HD], BF)
                wkv_sb = wp.tile([128, 32, 2 * HD], BF)
                # first x/table tiles ahead of the big weight DMAs so the
                # tensor engine starts as soon as the first weight rows land
                xt0 = px.tile([128, 32, 256], BF, tag="xt")
                nc.sync.dma_start(xt0[:], xT_r[:, :, 0:256])
                rcc0 = px.tile([128, 2, 5, 128], BF, tag="rcc")
                nc.sync.dma_start(rcc0[:], rcc_r[:, 0:2, :, :])
                rss0 = px.tile([128, 2, 5, 128], BF, tag="rss")
                nc.sync.dma_start(rss0[:], rss_r[:, 0:2, :, :])
                wchunks = [(0, 2), (2, 8), (8, 16), (16, 24), (24, 32)]
                for j0, j1 in wchunks:
                    nc.sync.dma_start(wq_sb[:, j0:j1, :], wq_r[:, j0:j1, :])
                    nc.sync.dma_start(wkv_sb[:, j0:j1, :], wkv_r[:, j0:j1, :])

                for i2 in range(NT // 2):
                    if i2 == 0:
                        xt, rcc, rss = xt0, rcc0, rss0
                    else:
                        xt = px.tile([128, 32, 256], BF, tag="xt")
                        nc.sync.dma_start(xt[:], xT_r[:, :, 256 * i2:256 * (i2 + 1)])
                        rcc = px.tile([128, 2, 5, 128], BF, tag="rcc")
                        nc.sync.dma_start(rcc[:], rcc_r[:, 2 * i2:2 * i2 + 2, :, :])
                        rss = px.tile([128, 2, 5, 128], BF, tag="rss")
                        nc.sync.dma_start(rss[:], rss_r[:, 2 * i2:2 * i2 + 2, :, :])
                    for ii in range(2):
                        i = 2 * i2 + ii
                        psq = ps1.tile([128, QH, HD], F32, tag="psq")
                        pskv = ps1.tile([128, 2, HD], F32, tag="pskv")
                        for j in range(32):
                            xs = xt[:, j, 128 * ii:128 * (ii + 1)]
                            nc.tensor.matmul(psq[:], xs, wq_sb[:, j, :],
                                             start=(j == 0), stop=(j == 31))
                            nc.tensor.matmul(pskv[:], xs, wkv_sb[:, j, :],
                                             start=(j == 0), stop=(j == 31))
                        # v to resident (DVE, casts to bf16)
                        nc.vector.tensor_copy(out=v_sb[:, i, :], in_=pskv[:, 1, :])
                        # combined q(4 heads)+k tile [128, 5, 128] in bf16
                        qkc = p1.tile([128, 5, 128], BF, tag="qkc")
                        nc.scalar.copy(qkc[:, 0:4, :], psq[:])
                        nc.scalar.copy(qkc[:, 4, :], pskv[:, 0, :])
                        # half-swapped copy for rope
                        qksw = p1.tile([128, 5, 128], BF, tag="qksw")
                        nc.scalar.copy(qksw[:, :, 0:64], qkc[:, :, 64:128])
                        nc.scalar.copy(qksw[:, :, 64:128], qkc[:, :, 0:64])
                        # rms stats
                        sq = p1.tile([128, 5, 128], BF, tag="sq")
                        nc.vector.tensor_tensor(sq[:], qkc[:], qkc[:], MUL)
                        ssq = p1.tile([128, 5, 1], F32, tag="ssq")
                        nc.vector.reduce_sum(ssq[:], sq[:], axis=X_AX)
                        var = p1.tile([128, 5, 1], F32, tag="var")
                        nc.vector.tensor_scalar(var[:], ssq[:], 1.0 / HD, EPS,
                                                MUL, ADD)
                        rms = p1.tile([128, 5, 1], F32, tag="rms")
                        nc.scalar.activation(rms[:], var[:], AF.Sqrt)
                        inv = p1.tile([128, 5, 1], F32, tag="inv")
                        nc.vector.reciprocal(inv[:], rms[:])
                        # rope (tables carry norm-w; q tables also softmax scale)
                        t1 = p1.tile([128, 5, 128], BF, tag="t1")
                        nc.vector.tensor_tensor(t1[:], qkc[:], rcc[:, ii], MUL)
                        t2 = p1.tile([128, 5, 128], BF, tag="t2")
                        nc.vector.tensor_tensor(t2[:], qksw[:], rss[:, ii], MUL)
                        qr = p1.tile([128, 5, 128], BF, tag="qr")
                        nc.vector.tensor_tensor(qr[:], t1[:], t2[:], ADD)
                        qn = p1.tile([128, 5, 128], BF, tag="qn")
                        for c5 in range(5):
                            nc.vector.tensor_scalar_mul(
                                qn[:, c5, :], qr[:, c5, :], inv[:, c5, :])
                        # transpose into resident [hd, tok] layouts
                        with nc.allow_low_precision(reason="pure transpose"):
                            for h in range(QH):
                                pt = pst.tile([128, 128], BF, tag="pt")
                                nc.tensor.transpose(pt[:], qn[:, h, :], ident_sb[:])
                                nc.vector.tensor_copy(
                                    out=qT_sb[:, h, 128 * i:128 * (i + 1)], in_=pt[:])
                            pt = pst.tile([128, 128], BF, tag="pt")
                            nc.tensor.transpose(pt[:], qn[:, 4, :], ident_sb[:])
                            nc.vector.tensor_copy(
                                out=kT_sb[:, 128 * i:128 * (i + 1)], in_=pt[:])

            # ---------------- phase 2: attention + per-head a2a -------------
            with (
                tc.tile_pool(name="p2", bufs=3) as p2,
                tc.tile_pool(name="ps2", bufs=2, space="PSUM") as ps2,
                tc.tile_pool(name="pso", bufs=2, space="PSUM") as pso,
                tc.tile_pool(name="psos", bufs=1, space="PSUM") as psos,
                tc.tile_pool(name="psb2", bufs=1, space="PSUM") as psbp,
            ):
                for h in range(QH):
                    for b in range(B):
                        for qb in range(4):
                            q0 = b * L + QB * qb
                            nkt = 4 * qb + 4
                            pso_o = pso.tile([128, QB], F32, tag="o")
                            pso_s = psos.tile([1, QB], F32, tag="s")
                            # paired k-tiles: two score matmuls into one
                            # 2-bank PSUM tile, a single exp over both
                            # (halves the scalar engine's per-inst gap);
                            # s/o matmuls of pair p-1 issue while exp(p)
                            # runs so the PE never waits on the scalar chain
                            pend = []

                            def flush_so(pkt, ppT_ap):
                                nc.tensor.matmul(pso_s[:], onesc_sb[:], ppT_ap,
                                                 start=(pkt == 0),
                                                 stop=(pkt == nkt - 1))
                                nc.tensor.matmul(pso_o[:], v_sb[:, b * 16 + pkt, :],
                                                 ppT_ap, start=(pkt == 0),
                                                 stop=(pkt == nkt - 1))

                            for kp in range(nkt // 2):
                                pss2 = ps2.tile([128, 2, QB], F32, tag="pss2")
                                for u in range(2):
                                    kt = 2 * kp + u
                                    kc_ = b * L + 128 * kt
                                    nc.tensor.matmul(pss2[:, u, :],
                                                     kT_sb[:, kc_:kc_ + 128],
                                                     qT_sb[:, h, q0:q0 + QB],
                                                     start=True, stop=True)
                                pT2 = p2.tile([128, 2, QB], BF, tag="pT2")
                                nc.scalar.activation(pT2[:], pss2[:], AF.Exp)
                                for u in range(2):
                                    kt = 2 * kp + u
                                    t = kt - 4 * qb
                                    if t >= 0:
                                        off = 384 - 128 * t
                                        nc.vector.tensor_tensor(
                                            pT2[:, u, :], pT2[:, u, :],
                                            pat_sb[:, off:off + QB], MUL)
                                pend.append((2 * kp, pT2))
                                if len(pend) > 1:
                                    pkp, ppT2 = pend.pop(0)
                                    flush_so(pkp, ppT2[:, 0, :])
                                    flush_so(pkp + 1, ppT2[:, 1, :])
                            pkp, ppT2 = pend.pop(0)
                            flush_so(pkp, ppT2[:, 0, :])
                            flush_so(pkp + 1, ppT2[:, 1, :])
                            rec = p2.tile([1, QB], F32, tag="rec")
                            nc.vector.reciprocal_approx_fast(rec[:], pso_s[:])
                            rec16 = p2.tile([1, QB], BF, tag="rec16")
                            nc.vector.tensor_copy(out=rec16[:], in_=rec[:])
                            psb = psbp.tile([128, QB], F32, tag="psb")
                            nc.tensor.matmul(psb[:], onesr_sb[:], rec16[:],
                                             start=True, stop=True)
                            bcs = p2.tile([128, QB], BF, tag="bcs")
                            nc.vector.tensor_copy(out=bcs[:], in_=psb[:])
                            attn = p2.tile([128, QB], BF, tag="attn")
                            nc.vector.tensor_tensor(attn[:], pso_o[:], bcs[:], MUL)
                            j = 4 * b + qb
                            nc.sync.dma_start(a2a_in[h][j, :, :], attn[:])
                    nc.gpsimd.collective_compute(
                        "AllToAll", mybir.AluOpType.bypass,
                        replica_groups=[list(range(NC_))],
                        ins=[a2a_in[h].opt()], outs=[a2a_out[h].opt()])

            # ---------------- phase 4: output projection --------------------
            with (
                tc.tile_pool(name="p4a", bufs=1) as p4a,
                tc.tile_pool(name="p4w", bufs=3) as p4w,
                tc.tile_pool(name="p4o", bufs=2) as p4o,
                tc.tile_pool(name="ps4", bufs=2, space="PSUM") as ps4,
            ):
                at = []
                for h in range(QH):
                    ath = p4a.tile([128, NC_, QB], BF, name=f"at{h}")
                    nc.sync.dma_start(
                        ath[:], a2a_out[h][:].rearrange("s p t -> p s t"))
                    at.append(ath)
                def emit_chunks(oc, pso_list, hs):
                    for h4 in hs:
                        wt = p4w.tile([128, 8, 512], BF, tag="wt")
                        nc.sync.dma_start(
                            wt[:], wo_r[:, 8 * h4:8 * (h4 + 1), 512 * oc:512 * (oc + 1)])
                        for s in range(8):
                            k = 8 * h4 + s
                            for tt in range(4):
                                nc.tensor.matmul(
                                    pso_list[tt][:],
                                    at[h4][:, s, 128 * tt:128 * (tt + 1)],
                                    wt[:, s, :], start=(k == 0), stop=(k == 31))

                def flush(oc, pso_list):
                    for tt in range(4):
                        ob = p4o.tile([128, 512], F32, tag="ob")
                        nc.vector.tensor_copy(out=ob[:], in_=pso_list[tt][:])
                        nc.sync.dma_start(
                            out[128 * tt:128 * (tt + 1), 512 * oc:512 * (oc + 1)], ob[:])

                # skewed: defer each oc's h3 chunk until after the next oc's
                # h0-h2, so the wait for the last a2a overlaps real work
                pend4 = None
                for oc in range(8):
                    pso_list = [ps4.tile([128, 512], F32, tag=f"po{tt}", name=f"po{tt}")
                                for tt in range(4)]
                    emit_chunks(oc, pso_list, [0, 1, 2])
                    if pend4 is not None:
                        emit_chunks(pend4[0], pend4[1], [3])
                        flush(*pend4)
                    pend4 = (oc, pso_list)
                emit_chunks(pend4[0], pend4[1], [3])
                flush(*pend4)

    nc.compile()
    return nc


def _prep(inputs):
    x = np.asarray(inputs["x"], np.float32)
    wq = np.asarray(inputs["wq"], np.float32)
    wk = np.asarray(inputs["wk"], np.float32)
    wv = np.asarray(inputs["wv"], np.float32)
    wo = np.asarray(inputs["wo"], np.float32)
    qw = np.asarray(inputs["q_norm_w"], np.float32)
    kw = np.asarray(inputs["k_norm_w"], np.float32)

    xT = np.ascontiguousarray(x.reshape(T, D).T).astype(BF16NP)

    half = HD // 2
    inv_freq = 1.0 / (ROPE_BASE ** (np.arange(half, dtype=np.float32) / half))
    pos = np.arange(L, dtype=np.float32)
    ang = pos[:, None] * inv_freq[None, :]
    cos = np.cos(ang).astype(np.float32)
    sin = np.sin(ang).astype(np.float32)
    scale = np.float32(HD ** -0.5)

    def rope_tabs(w, s):
        # out[j] = qk[j]*cc[j] + qksw[j]*ss[j]   (qksw = half-swapped qk)
        cc = np.concatenate([cos * w[None, 0:half], cos * w[None, half:HD]], axis=1) * s
        ss = np.concatenate([-sin * w[None, half:HD], sin * w[None, 0:half]], axis=1) * s
        return cc, ss  # each [L, HD]

    ccq, ssq_ = rope_tabs(qw, scale)
    cck, ssk = rope_tabs(kw, np.float32(1.0))
    ropecc = np.concatenate([ccq, ccq, ccq, ccq, cck], axis=1)   # [L, 640]
    ropess = np.concatenate([ssq_, ssq_, ssq_, ssq_, ssk], axis=1)
    ropecc = np.ascontiguousarray(np.tile(ropecc, (B, 1))).astype(BF16NP)
    ropess = np.ascontiguousarray(np.tile(ropess, (B, 1))).astype(BF16NP)

    kk = np.arange(128)[:, None]
    cc_ = np.arange(896)[None, :]
    pat = (kk <= cc_ - 384).astype(BF16NP)
    ident = np.eye(128, dtype=BF16NP)
    ones_c = np.ones((128, 1), BF16NP)
    ones_r = np.ones((1, 128), BF16NP)

    # wo rows permuted to (h_local, src_core, hd) chunk order
    perm = np.concatenate([
        np.arange(128) + (s * QH + h4) * 128
        for h4 in range(QH) for s in range(NC_)])
    wo_p = np.ascontiguousarray(wo[perm, :]).astype(BF16NP)

    in_maps = []
    for c in range(NC_):
        in_maps.append({
            "xT": xT,
            "wq": np.ascontiguousarray(wq[:, 512 * c:512 * (c + 1)]).astype(BF16NP),
            "wkv": np.ascontiguousarray(np.concatenate(
                [wk[:, HD * c:HD * (c + 1)], wv[:, HD * c:HD * (c + 1)]],
                axis=1)).astype(BF16NP),
            "wo": wo_p,
            "ropecc": ropecc,
            "ropess": ropess,
            "pat": pat,
            "ident": ident,
            "ones_c": ones_c,
            "ones_r": ones_r,
        })
    return in_maps


def kernel(**inputs) -> np.ndarray:
    if "nc" not in _CACHE:
        _CACHE["nc"] = _build()
    nc = _CACHE["nc"]
    in_maps = _prep(inputs)
    res = run_bass_kernel_spmd(nc, in_maps, list(range(NC_)))
    chunks = [res.results[c]["out"] for c in range(NC_)]
    return np.concatenate(chunks, axis=0).reshape(B, L, D)


# revision 17
# speedup vs baseline: 1.2254x; 1.0561x over previous
"""GQA attention layer (B=2, L=2048, D=4096, H=32, KH=8, HD=128) on 8 TRN2 cores.

Sharding: tensor-parallel over KV heads (1 kv head + 4 q heads per core).
All matmul operands in bf16 (fp32 streams at half rate on TRN2 PE and gets no
FWL weight loads). Per core: QKV projection (x^T-tile stationary), fused
per-head RMSNorm + RoPE via host-precomputed cc/ss tables (norm weight and
softmax scale folded in), PE transposes into resident SBUF qT/kT, flash-style
attention in S^T layout with causal block skipping, per-head AllToAll (4
collectives, pipelined against attention + output projection), then the output
projection for this core's 512-token slice. Host assembles the 8 token slices.
"""
import numpy as np
import ml_dtypes

import concourse.bass as bass
import concourse.mybir as mybir
import concourse.tile as tile
from concourse import bacc
from concourse.bass_utils import run_bass_kernel_spmd

F32 = mybir.dt.float32
F32R = mybir.dt.float32r
BF = mybir.dt.bfloat16
BF16NP = ml_dtypes.bfloat16
AF = mybir.ActivationFunctionType
MUL = mybir.AluOpType.mult
ADD = mybir.AluOpType.add
X_AX = mybir.AxisListType.X

B, L, D = 2, 2048, 4096
H, KH, HD = 32, 8, 128
T = B * L              # 4096 tokens
NC_ = 8                # cores
QH = H // NC_          # 4 q heads per core
QB = 512               # q block
NT = T // 128          # 32 token tiles
EPS = 1e-5
ROPE_BASE = 1000000.0

_CACHE = {}


def _build():
    nc = bacc.Bacc("TRN2", target_bir_lowering=False, debug=False, num_devices=NC_)

    xT = nc.dram_tensor("xT", [D, T], BF, kind="ExternalInput").ap()
    wq = nc.dram_tensor("wq", [D, QH * HD], BF, kind="ExternalInput").ap()
    wkv = nc.dram_tensor("wkv", [D, 2 * HD], BF, kind="ExternalInput").ap()
    wo = nc.dram_tensor("wo", [D, D], BF, kind="ExternalInput").ap()
    ropecc = nc.dram_tensor("ropecc", [T, 5 * HD], BF, kind="ExternalInput").ap()
    ropess = nc.dram_tensor("ropess", [T, 5 * HD], BF, kind="ExternalInput").ap()
    pat = nc.dram_tensor("pat", [128, 896], BF, kind="ExternalInput").ap()
    ident = nc.dram_tensor("ident", [128, 128], BF, kind="ExternalInput").ap()
    ones_c = nc.dram_tensor("ones_c", [128, 1], BF, kind="ExternalInput").ap()
    ones_r = nc.dram_tensor("ones_r", [1, 128], BF, kind="ExternalInput").ap()
    out = nc.dram_tensor("out", [T // NC_, D], F32, kind="ExternalOutput").ap()

    xT_r = xT.rearrange("(o p) t -> p o t", p=128)         # [128, 32, T]
    wq_r = wq.rearrange("(o p) n -> p o n", p=128)         # [128, 32, 512]
    wkv_r = wkv.rearrange("(o p) n -> p o n", p=128)       # [128, 32, 256]
    wo_r = wo.rearrange("(o p) n -> p o n", p=128)         # [128, 32, 4096]
    rcc_r = ropecc.rearrange("(n p) (c j) -> p n c j", p=128, j=128)  # [128,32,5,128]
    rss_r = ropess.rearrange("(n p) (c j) -> p n c j", p=128, j=128)

    with tile.TileContext(nc) as tc:
        with (
            tc.tile_pool(name="const", bufs=1) as cp,
            tc.tile_pool(name="dram", bufs=1, space="DRAM") as dramp,
            tc.tile_pool(name="res", bufs=1) as resp,
        ):
            ident_sb = cp.tile([128, 128], BF)
            nc.sync.dma_start(ident_sb[:], ident)
            pat_sb = cp.tile([128, 896], BF)
            nc.sync.dma_start(pat_sb[:], pat)
            onesc_sb = cp.tile([128, 1], BF)
            nc.sync.dma_start(onesc_sb[:], ones_c)
            onesr_sb = cp.tile([1, 128], BF)
            nc.sync.dma_start(onesr_sb[:], ones_r)

            kT_sb = resp.tile([128, T], BF)                # [hd, tok]
            v_sb = resp.tile([128, NT, HD], BF)            # [tok%128, tile, hd]
            qT_sb = resp.tile([128, QH, T], BF)            # [hd, head, tok]

            a2a_in = [dramp.tile([NC_, HD, QB], BF, name=f"a2a_in{h}")
                      for h in range(QH)]
            a2a_out = [dramp.tile([NC_, HD, QB], BF, name=f"a2a_out{h}")
                       for h in range(QH)]

            # ---------------- phase 1: projections + norm + rope ----------
            with (
                tc.tile_pool(name="wts", bufs=1) as wp,
                tc.tile_pool(name="px", bufs=2) as px,
                tc.tile_pool(name="p1", bufs=2) as p1,
                tc.tile_pool(name="ps1", bufs=3, space="PSUM") as ps1,
                tc.tile_pool(name="pst", bufs=2, space="PSUM") as pst,
            ):
                wq_sb = wp.tile([128, 32, QH * HD], BF)
                wkv_sb = wp.tile([128, 32, 2 * HD], BF)
                # first x/table tiles ahead of the big weight DMAs so the
                # tensor engine starts as soon as the first weight rows land
                xt0 = px.tile([128, 32, 256], BF, tag="xt")
                nc.sync.dma_start(xt0[:], xT_r[:, :, 0:256])
                rcc0 = px.tile([128, 2, 5, 128], BF, tag="rcc")
                nc.sync.dma_start(rcc0[:], rcc_r[:, 0:2, :, :])
                rss0 = px.tile([128, 2, 5, 128], BF, tag="rss")
                nc.sync.dma_start(rss0[:], rss_r[:, 0:2, :, :])
                wchunks = [(0, 2), (2, 8), (8, 16), (16, 24), (24, 32)]
                for j0, j1 in wchunks:
                    nc.sync.dma_start(wq_sb[:, j0:j1, :], wq_r[:, j0:j1, :])
                    nc.sync.dma_start(wkv_sb[:, j0:j1, :], wkv_r[:, j0:j1, :])

                for i2 in range(NT // 2):
                    if i2 == 0:
                        xt, rcc, rss = xt0, rcc0, rss0
                    else:
                        xt = px.tile([128, 32, 256], BF, tag="xt")
                        nc.sync.dma_start(xt[:], xT_r[:, :, 256 * i2:256 * (i2 + 1)])
                        rcc = px.tile([128, 2, 5, 128], BF, tag="rcc")
                        nc.sync.dma_start(rcc[:], rcc_r[:, 2 * i2:2 * i2 + 2, :, :])
                        rss = px.tile([128, 2, 5, 128], BF, tag="rss")
                        nc.sync.dma_start(rss[:], rss_r[:, 2 * i2:2 * i2 + 2, :, :])
                    for ii in range(2):
                        i = 2 * i2 + ii
                        psq = ps1.tile([128, QH, HD], F32, tag="psq")
                        pskv = ps1.tile([128, 2, HD], F32, tag="pskv")
                        for j in range(32):
                            xs = xt[:, j, 128 * ii:128 * (ii + 1)]
                            nc.tensor.matmul(psq[:], xs, wq_sb[:, j, :],
                                             start=(j == 0), stop=(j == 31))
                            nc.tensor.matmul(pskv[:], xs, wkv_sb[:, j, :],
                                             start=(j == 0), stop=(j == 31))
                        # v to resident (DVE, casts to bf16)
                        nc.vector.tensor_copy(out=v_sb[:, i, :], in_=pskv[:, 1, :])
                        # combined q(4 heads)+k tile [128, 5, 128] in bf16
                        qkc = p1.tile([128, 5, 128], BF, tag="qkc")
                        nc.scalar.copy(qkc[:, 0:4, :], psq[:])
                        nc.scalar.copy(qkc[:, 4, :], pskv[:, 0, :])
                        # half-swapped copy for rope
                        qksw = p1.tile([128, 5, 128], BF, tag="qksw")
                        nc.scalar.copy(qksw[:, :, 0:64], qkc[:, :, 64:128])
                        nc.scalar.copy(qksw[:, :, 64:128], qkc[:, :, 0:64])
                        # rms stats
                        sq = p1.tile([128, 5, 128], BF, tag="sq")
                        nc.vector.tensor_tensor(sq[:], qkc[:], qkc[:], MUL)
                        ssq = p1.tile([128, 5, 1], F32, tag="ssq")
                        nc.vector.reduce_sum(ssq[:], sq[:], axis=X_AX)
                        var = p1.tile([128, 5, 1], F32, tag="var")
                        nc.vector.tensor_scalar(var[:], ssq[:], 1.0 / HD, EPS,
                                                MUL, ADD)
                        rms = p1.tile([128, 5, 1], F32, tag="rms")
                        nc.scalar.activation(rms[:], var[:], AF.Sqrt)
                        inv = p1.tile([128, 5, 1], F32, tag="inv")
                        nc.vector.reciprocal(inv[:], rms[:])
                        # rope (tables carry norm-w; q tables also softmax scale)
                        t1 = p1.tile([128, 5, 128], BF, tag="t1")
                        nc.vector.tensor_tensor(t1[:], qkc[:], rcc[:, ii], MUL)
                        t2 = p1.tile([128, 5, 128], BF, tag="t2")
                        nc.vector.tensor_tensor(t2[:], qksw[:], rss[:, ii], MUL)
                        qr = p1.tile([128, 5, 128], BF, tag="qr")
                        nc.vector.tensor_tensor(qr[:], t1[:], t2[:], ADD)
                        qn = p1.tile([128, 5, 128], BF, tag="qn")
                        for c5 in range(5):
                            nc.vector.tensor_scalar_mul(
                                qn[:, c5, :], qr[:, c5, :], inv[:, c5, :])
                        # transpose into resident [hd, tok] layouts
                        with nc.allow_low_precision(reason="pure transpose"):
                            for h in range(QH):
                                pt = pst.tile([128, 128], BF, tag="pt")
                                nc.tensor.transpose(pt[:], qn[:, h, :], ident_sb[:])
                                nc.vector.tensor_copy(
                                    out=qT_sb[:, h, 128 * i:128 * (i + 1)], in_=pt[:])
                            pt = pst.tile([128, 128], BF, tag="pt")
                            nc.tensor.transpose(pt[:], qn[:, 4, :], ident_sb[:])
                            nc.vector.tensor_copy(
                                out=kT_sb[:, 128 * i:128 * (i + 1)], in_=pt[:])

            # ---------------- phase 2: attention + per-head a2a -------------
            with (
                tc.tile_pool(name="p2", bufs=3) as p2,
                tc.tile_pool(name="ps2", bufs=2, space="PSUM") as ps2,
                tc.tile_pool(name="pso", bufs=2, space="PSUM") as pso,
                tc.tile_pool(name="psos", bufs=1, space="PSUM") as psos,
                tc.tile_pool(name="psb2", bufs=1, space="PSUM") as psbp,
            ):
                for h in range(QH):
                    for b in range(B):
                        for qb in range(4):
                            q0 = b * L + QB * qb
                            nkt = 4 * qb + 4
                            pso_o = pso.tile([128, QB], F32, tag="o")
                            pso_s = psos.tile([1, QB], F32, tag="s")
                            acc = p2.tile([128, QB], BF, tag="acc")
                            # paired k-tiles: two score matmuls into one
                            # 2-bank PSUM tile, a single exp over both
                            # (halves the scalar engine's per-inst gap);
                            # o-matmuls of pair p-1 issue while exp(p) runs.
                            # The softmax denominator accumulates on the DVE
                            # (bf16) with one ones-matmul per block, keeping
                            # the PE to 4 matmuls per pair instead of 6.
                            pend = []

                            def flush_o(pkt, ppT_ap):
                                nc.tensor.matmul(pso_o[:], v_sb[:, b * 16 + pkt, :],
                                                 ppT_ap, start=(pkt == 0),
                                                 stop=(pkt == nkt - 1))

                            for kp in range(nkt // 2):
                                pss2 = ps2.tile([128, 2, QB], F32, tag="pss2")
                                for u in range(2):
                                    kt = 2 * kp + u
                                    kc_ = b * L + 128 * kt
                                    nc.tensor.matmul(pss2[:, u, :],
                                                     kT_sb[:, kc_:kc_ + 128],
                                                     qT_sb[:, h, q0:q0 + QB],
                                                     start=True, stop=True)
                                pT2 = p2.tile([128, 2, QB], BF, tag="pT2")
                                nc.scalar.activation(pT2[:], pss2[:], AF.Exp)
                                for u in range(2):
                                    kt = 2 * kp + u
                                    t = kt - 4 * qb
                                    if t >= 0:
                                        off = 384 - 128 * t
                                        nc.vector.tensor_tensor(
                                            pT2[:, u, :], pT2[:, u, :],
                                            pat_sb[:, off:off + QB], MUL)
                                    if kt == 0:
                                        nc.vector.tensor_copy(out=acc[:],
                                                              in_=pT2[:, 0, :])
                                    else:
                                        with nc.allow_low_precision(
                                                reason="bf16 softmax denom"):
                                            nc.vector.tensor_tensor(
                                                acc[:], acc[:], pT2[:, u, :], ADD)
                                pend.append((2 * kp, pT2))
                                if len(pend) > 1:
                                    pkp, ppT2 = pend.pop(0)
                                    flush_o(pkp, ppT2[:, 0, :])
                                    flush_o(pkp + 1, ppT2[:, 1, :])
                            pkp, ppT2 = pend.pop(0)
                            flush_o(pkp, ppT2[:, 0, :])
                            flush_o(pkp + 1, ppT2[:, 1, :])
                            nc.tensor.matmul(pso_s[:], onesc_sb[:], acc[:],
                                             start=True, stop=True)
                            rec = p2.tile([1, QB], F32, tag="rec")
                            nc.vector.reciprocal_approx_fast(rec[:], pso_s[:])
                            rec16 = p2.tile([1, QB], BF, tag="rec16")
                            nc.vector.tensor_copy(out=rec16[:], in_=rec[:])
                            psb = psbp.tile([128, QB], F32, tag="psb")
                            nc.tensor.matmul(psb[:], onesr_sb[:], rec16[:],
                                             start=True, stop=True)
                            bcs = p2.tile([128, QB], BF, tag="bcs")
                            nc.vector.tensor_copy(out=bcs[:], in_=psb[:])
                            attn = p2.tile([128, QB], BF, tag="attn")
                            nc.vector.tensor_tensor(attn[:], pso_o[:], bcs[:], MUL)
                            j = 4 * b + qb
                            nc.sync.dma_start(a2a_in[h][j, :, :], attn[:])
                    nc.gpsimd.collective_compute(
                        "AllToAll", mybir.AluOpType.bypass,
                        replica_groups=[list(range(NC_))],
                        ins=[a2a_in[h].opt()], outs=[a2a_out[h].opt()])

            # ---------------- phase 4: output projection --------------------
            with (
                tc.tile_pool(name="p4a", bufs=1) as p4a,
                tc.tile_pool(name="p4w", bufs=3) as p4w,
                tc.tile_pool(name="p4o", bufs=2) as p4o,
                tc.tile_pool(name="ps4", bufs=2, space="PSUM") as ps4,
            ):
                at = []
                for h in range(QH):
                    ath = p4a.tile([128, NC_, QB], BF, name=f"at{h}")
                    nc.sync.dma_start(
                        ath[:], a2a_out[h][:].rearrange("s p t -> p s t"))
                    at.append(ath)
                def emit_chunks(oc, pso_list, hs):
                    for h4 in hs:
                        wt = p4w.tile([128, 8, 512], BF, tag="wt")
                        nc.sync.dma_start(
                            wt[:], wo_r[:, 8 * h4:8 * (h4 + 1), 512 * oc:512 * (oc + 1)])
                        for s in range(8):
                            k = 8 * h4 + s
                            for tt in range(4):
                                nc.tensor.matmul(
                                    pso_list[tt][:],
                                    at[h4][:, s, 128 * tt:128 * (tt + 1)],
                                    wt[:, s, :], start=(k == 0), stop=(k == 31))

                def flush(oc, pso_list):
                    for tt in range(4):
                        ob = p4o.tile([128, 512], F32, tag="ob")
                        nc.vector.tensor_copy(out=ob[:], in_=pso_list[tt][:])
                        nc.sync.dma_start(
                            out[128 * tt:128 * (tt + 1), 512 * oc:512 * (oc + 1)], ob[:])

                # skewed: defer each oc's h3 chunk until after the next oc's
                # h0-h2, so the wait for the last a2a overlaps real work
                pend4 = None
                for oc in range(8):
                    pso_list = [ps4.tile([128, 512], F32, tag=f"po{tt}", name=f"po{tt}")
                                for tt in range(4)]
                    emit_chunks(oc, pso_list, [0, 1, 2])
                    if pend4 is not None:
                        emit_chunks(pend4[0], pend4[1], [3])
                        flush(*pend4)
                    pend4 = (oc, pso_list)
                emit_chunks(pend4[0], pend4[1], [3])
                flush(*pend4)

    nc.compile()
    return nc


def _prep(inputs):
    x = np.asarray(inputs["x"], np.float32)
    wq = np.asarray(inputs["wq"], np.float32)
    wk = np.asarray(inputs["wk"], np.float32)
    wv = np.asarray(inputs["wv"], np.float32)
    wo = np.asarray(inputs["wo"], np.float32)
    qw = np.asarray(inputs["q_norm_w"], np.float32)
    kw = np.asarray(inputs["k_norm_w"], np.float32)

    xT = np.ascontiguousarray(x.reshape(T, D).T).astype(BF16NP)

    half = HD // 2
    inv_freq = 1.0 / (ROPE_BASE ** (np.arange(half, dtype=np.float32) / half))
    pos = np.arange(L, dtype=np.float32)
    ang = pos[:, None] * inv_freq[None, :]
    cos = np.cos(ang).astype(np.float32)
    sin = np.sin(ang).astype(np.float32)
    scale = np.float32(HD ** -0.5)

    def rope_tabs(w, s):
        # out[j] = qk[j]*cc[j] + qksw[j]*ss[j]   (qksw = half-swapped qk)
        cc = np.concatenate([cos * w[None, 0:half], cos * w[None, half:HD]], axis=1) * s
        ss = np.concatenate([-sin * w[None, half:HD], sin * w[None, 0:half]], axis=1) * s
        return cc, ss  # each [L, HD]

    ccq, ssq_ = rope_tabs(qw, scale)
    cck, ssk = rope_tabs(kw, np.float32(1.0))
    ropecc = np.concatenate([ccq, ccq, ccq, ccq, cck], axis=1)   # [L, 640]
    ropess = np.concatenate([ssq_, ssq_, ssq_, ssq_, ssk], axis=1)
    ropecc = np.ascontiguousarray(np.tile(ropecc, (B, 1))).astype(BF16NP)
    ropess = np.ascontiguousarray(np.tile(ropess, (B, 1))).astype(BF16NP)

    kk = np.arange(128)[:, None]
    cc_ = np.arange(896)[None, :]
    pat = (kk <= cc_ - 384).astype(BF16NP)
    ident = np.eye(128, dtype=BF16NP)
    ones_c = np.ones((128, 1), BF16NP)
    ones_r = np.ones((1, 128), BF16NP)

    # wo rows permuted to (h_local, src_core, hd) chunk order
    perm = np.concatenate([
        np.arange(128) + (s * QH + h4) * 128
        for h4 in range(QH) for s in range(NC_)])
    wo_p = np.ascontiguousarray(wo[perm, :]).astype(BF16NP)

    in_maps = []
    for c in range(NC_):
        in_maps.append({
            "xT": xT,
            "wq": np.ascontiguousarray(wq[:, 512 * c:512 * (c + 1)]).astype(BF16NP),
            "wkv": np.ascontiguousarray(np.concatenate(
                [wk[:, HD * c:HD * (c + 1)], wv[:, HD * c:HD * (c + 1)]],
                axis=1)).astype(BF16NP),
            "wo": wo_p,
            "ropecc": ropecc,
            "ropess": ropess,
            "pat": pat,
            "ident": ident,
            "ones_c": ones_c,
            "ones_r": ones_r,
        })
    return in_maps


def kernel(**inputs) -> np.ndarray:
    if "nc" not in _CACHE:
        _CACHE["nc"] = _build()
    nc = _CACHE["nc"]
    in_maps = _prep(inputs)
    res = run_bass_kernel_spmd(nc, in_maps, list(range(NC_)))
    chunks = [res.results[c]["out"] for c in range(NC_)]
    return np.concatenate(chunks, axis=0).reshape(B, L, D)
